# revision 2
# baseline (speedup 1.0000x reference)
"""2D Haar DWT (analysis) on 8 Trainium2 NeuronCores.

Input  x: (16, 64, 256, 256) f32  -> 1024 independent 256x256 images.
Output: tuple (LL, LH, HL, HH), each (16, 64, 128, 128) f32.

Math per image (reference):
    L  = m_l0 @ x          # [128,256] row low-pass
    Hh = m_h0 @ x
    LL = L @ m_l1, LH = L @ m_h1, HL = Hh @ m_l1, HH = Hh @ m_h1

Device formulation (TensorE computes out = lhsT.T @ rhs; lhsT/rhs need the
contraction dim on partitions, so we keep x un-transposed by computing
    T1 = x.T @ [m_l0.T | m_h0.T]        # [256, 256] = [L.T | Hh.T]
    out = T1cols.T @ [m_l1 | m_h1]      # [128, 512] = [LL|LH|HL|HH]
which needs no on-chip transposes at all).

Sharding: data-parallel over the 1024 images, 128 per core. The two 256x256
combined wavelet matrices are replicated. Input/output DRAM layouts are
pre/post-permuted on host so every DMA moves 1-4 MiB of fully contiguous
per-partition data.
"""

import numpy as np

import concourse.bacc as bacc
import concourse.tile as tile
from concourse import mybir
from concourse.bass_utils import run_bass_kernel_spmd

N_CORES = 8
B, C, H, W = 16, 64, 256, 256
N_IMG = B * C                  # 1024
IMG_PER_CORE = N_IMG // N_CORES  # 128
G = 8                          # images per DMA group
NG = IMG_PER_CORE // G         # 16 groups per core
F32 = mybir.dt.float32

_CACHE = {}


def _build_program():
    nc = bacc.Bacc(
        "TRN2",
        target_bir_lowering=False,
        debug=False,
        enable_asserts=False,
        num_devices=N_CORES,
    )
    xin = nc.dram_tensor("xin", [NG, 2, 128, G, W], F32, kind="ExternalInput").ap()
    m1 = nc.dram_tensor("m1", [2, 128, 256], F32, kind="ExternalInput").ap()
    m2 = nc.dram_tensor("m2", [2, 128, 256], F32, kind="ExternalInput").ap()
    out = nc.dram_tensor("out", [NG, 128, G, 4, 128], F32, kind="ExternalOutput").ap()

    with tile.TileContext(nc) as tc:
        with (
            tc.tile_pool(name="consts", bufs=1) as consts,
            tc.tile_pool(name="xp", bufs=3) as xp,
            tc.tile_pool(name="t1sp", bufs=4) as t1sp,
            tc.tile_pool(name="op", bufs=3) as op,
            tc.tile_pool(name="p1", bufs=2, space="PSUM") as p1,
            tc.tile_pool(name="p2", bufs=2, space="PSUM") as p2,
        ):
            m1a = consts.tile([128, 256], F32, tag="m1a")
            m1b = consts.tile([128, 256], F32, tag="m1b")
            m2a = consts.tile([128, 256], F32, tag="m2a")
            m2b = consts.tile([128, 256], F32, tag="m2b")
            nc.sync.dma_start(out=m1a, in_=m1[0])
            nc.sync.dma_start(out=m1b, in_=m1[1])
            nc.sync.dma_start(out=m2a, in_=m2[0])
            nc.sync.dma_start(out=m2b, in_=m2[1])

            for g in range(NG):
                xa = xp.tile([128, G, W], F32, tag="xa")  # x rows 0:128
                xb = xp.tile([128, G, W], F32, tag="xb")  # x rows 128:256
                nc.sync.dma_start(out=xa, in_=xin[g, 0])
                nc.sync.dma_start(out=xb, in_=xin[g, 1])
                ot = op.tile([128, G, 512], F32, tag="ot")
                for i in range(G):
                    # stage 1: T1 = x_i.T @ [m_l0.T | m_h0.T]  -> [256, 256]
                    # packed as one PSUM bank: [:, 0:256] = T1 rows 0:128 (W),
                    # [:, 256:512] = T1 rows 128:256.
                    t1 = p1.tile([128, 512], F32, tag="t1")
                    nc.tensor.matmul(t1[:, 0:256], lhsT=xa[:, i, 0:128],
                                     rhs=m1a, start=True, stop=False)
                    nc.tensor.matmul(t1[:, 0:256], lhsT=xb[:, i, 0:128],
                                     rhs=m1b, start=False, stop=True)
                    nc.tensor.matmul(t1[:, 256:512], lhsT=xa[:, i, 128:256],
                                     rhs=m1a, start=True, stop=False)
                    nc.tensor.matmul(t1[:, 256:512], lhsT=xb[:, i, 128:256],
                                     rhs=m1b, start=False, stop=True)
                    t1s = t1sp.tile([128, 512], F32, tag="t1s")
                    nc.scalar.copy(t1s, t1)
                    # stage 2: [LL|LH] = L @ [m_l1|m_h1], [HL|HH] = Hh @ ...
                    o2 = p2.tile([128, 512], F32, tag="o2")
                    nc.tensor.matmul(o2[:, 0:256], lhsT=t1s[:, 0:128],
                                     rhs=m2a, start=True, stop=False)
                    nc.tensor.matmul(o2[:, 0:256], lhsT=t1s[:, 256:384],
                                     rhs=m2b, start=False, stop=True)
                    nc.tensor.matmul(o2[:, 256:512], lhsT=t1s[:, 128:256],
                                     rhs=m2a, start=True, stop=False)
                    nc.tensor.matmul(o2[:, 256:512], lhsT=t1s[:, 384:512],
                                     rhs=m2b, start=False, stop=True)
                    nc.vector.tensor_copy(out=ot[:, i], in_=o2)
                nc.sync.dma_start(out=out[g], in_=ot)
    nc.compile()
    return nc


def kernel(x, m_l0, m_l1, m_h0, m_h1):
    x = np.asarray(x, dtype=np.float32)
    assert x.shape == (B, C, H, W), x.shape

    # combined matrices, replicated to all cores; half k of M1/M2 is rows
    # k*128:(k+1)*128 of the [256, 256] concatenated matrix
    ml0T = np.asarray(m_l0, np.float32).T  # [256, 128]
    mh0T = np.asarray(m_h0, np.float32).T  # [256, 128]
    M1full = np.concatenate([ml0T, mh0T], axis=1)  # [256, 256]
    M1 = np.ascontiguousarray(M1full.reshape(2, 128, 256))
    M2full = np.concatenate([np.asarray(m_l1, np.float32),
                             np.asarray(m_h1, np.float32)], axis=1)  # [256, 256]
    M2 = np.ascontiguousarray(M2full.reshape(2, 128, 256))

    if "nc" not in _CACHE:
        _CACHE["nc"] = _build_program()
    nc = _CACHE["nc"]

    x2 = x.reshape(N_IMG, H, W)
    in_maps = []
    for s in range(N_CORES):
        shard = x2[s * IMG_PER_CORE:(s + 1) * IMG_PER_CORE]  # [128, 256, 256]
        arr = shard.reshape(NG, G, 2, 128, W).transpose(0, 2, 3, 1, 4)
        in_maps.append({
            "xin": np.ascontiguousarray(arr),
            "m1": M1,
            "m2": M2,
        })

    res = run_bass_kernel_spmd(nc, in_maps, core_ids=list(range(N_CORES)))

    parts = []
    for s in range(N_CORES):
        o = res.results[s]["out"]  # [NG, 128, G, 4, 128] = (g, h, i, band, w)
        o = o.transpose(0, 2, 3, 1, 4)  # (g, i, band, h, w)
        parts.append(o.reshape(IMG_PER_CORE, 4, H // 2, W // 2))
    full = np.concatenate(parts, axis=0).reshape(B, C, 4, H // 2, W // 2)
    LL = np.ascontiguousarray(full[:, :, 0])
    LH = np.ascontiguousarray(full[:, :, 1])
    HL = np.ascontiguousarray(full[:, :, 2])
    HH = np.ascontiguousarray(full[:, :, 3])
    return (LL, LH, HL, HH)


# revision 3
# speedup vs baseline: 2.5591x; 2.5591x over previous
"""2D Haar DWT (analysis) on 8 Trainium2 NeuronCores.

Input  x: (16, 64, 256, 256) f32  -> 1024 independent 256x256 images.
Output: tuple (LL, LH, HL, HH), each (16, 64, 128, 128) f32.

With Haar filters the DWT is a 2x2 butterfly: for each 2x2 block
(a b / c d), with s = 0.5:
    LL = s(a+b+c+d), LH = s(a-b+c-d), HL = s(a+b-c-d), HH = s(a-b-c+d)
which is two levels of adds/subs -- no matmul needed. fp32 matmuls run at
half rate on the PE and dominate; plain VectorE adds (1 elem/lane/cyc)
finish in ~150us/core, under the ~187us HBM roofline for 67MB of traffic.

Device layout (everything unit-stride, partition dim = image):
  - host prescales x by 0.5 and deinterleaves even/odd columns so the
    column-pair butterfly is two big contiguous tensor ops
  - per core: 128 images; 16 chunks of 16 image rows; per chunk one 2MB
    input DMA, 6 VectorE ops, one 2MB output DMA.
"""

import numpy as np

import concourse.bacc as bacc
import concourse.tile as tile
from concourse import mybir
from concourse.bass_utils import run_bass_kernel_spmd

N_CORES = 8
B, C, H, W = 16, 64, 256, 256
N_IMG = B * C                    # 1024
IMG_PER_CORE = N_IMG // N_CORES  # 128
HC = 16                          # image rows per chunk
NCH = H // HC                    # 16 chunks
F32 = mybir.dt.float32

_CACHE = {}


def _build_program():
    nc = bacc.Bacc(
        "TRN2",
        target_bir_lowering=False,
        debug=False,
        enable_asserts=False,
        num_devices=N_CORES,
    )
    # xin[k] = [img, h(16), e(2), w'(128)]: prescaled, even/odd-column split
    xin = nc.dram_tensor(
        "xin", [NCH, IMG_PER_CORE, HC, 2, W // 2], F32, kind="ExternalInput"
    ).ap()
    # out[k] = [img, band(4), lh(8), w'(128)]
    out = nc.dram_tensor(
        "out", [NCH, IMG_PER_CORE, 4, HC // 2, W // 2], F32, kind="ExternalOutput"
    ).ap()

    with tile.TileContext(nc) as tc:
        with (
            tc.tile_pool(name="xp", bufs=3) as xp,
            tc.tile_pool(name="mid", bufs=2) as mid,
            tc.tile_pool(name="op", bufs=3) as op,
        ):
            for k in range(NCH):
                xt = xp.tile([IMG_PER_CORE, HC, 2, W // 2], F32, tag="xt")
                nc.sync.dma_start(out=xt, in_=xin[k])
                # column butterfly: sw/dw[h] = x[h, even] +/- x[h, odd]
                sw = mid.tile([IMG_PER_CORE, HC // 2, 2, W // 2], F32, tag="sw")
                dw = mid.tile([IMG_PER_CORE, HC // 2, 2, W // 2], F32, tag="dw")
                xe = xt[:, :, 0, :].rearrange("p (i e) w -> p i e w", e=2)
                xo = xt[:, :, 1, :].rearrange("p (i e) w -> p i e w", e=2)
                nc.vector.tensor_add(sw, xe, xo)
                nc.vector.tensor_sub(dw, xe, xo)
                # row butterfly over adjacent rows -> 4 bands
                ot = op.tile([IMG_PER_CORE, 4, HC // 2, W // 2], F32, tag="ot")
                nc.vector.tensor_add(ot[:, 0], sw[:, :, 0, :], sw[:, :, 1, :])  # LL
                nc.vector.tensor_add(ot[:, 1], dw[:, :, 0, :], dw[:, :, 1, :])  # LH
                nc.vector.tensor_sub(ot[:, 2], sw[:, :, 0, :], sw[:, :, 1, :])  # HL
                nc.vector.tensor_sub(ot[:, 3], dw[:, :, 0, :], dw[:, :, 1, :])  # HH
                nc.scalar.dma_start(out=out[k], in_=ot)
    nc.compile()
    return nc


def kernel(x, m_l0, m_l1, m_h0, m_h1):
    x = np.asarray(x, dtype=np.float32)
    assert x.shape == (B, C, H, W), x.shape

    if "nc" not in _CACHE:
        _CACHE["nc"] = _build_program()
    nc = _CACHE["nc"]

    # [N, H, 2, W/2]: even/odd column split, prescaled by 0.5 (exact in fp32)
    xs = (x.reshape(N_IMG, H, W // 2, 2) * np.float32(0.5)).transpose(0, 1, 3, 2)
    in_maps = []
    for s in range(N_CORES):
        shard = xs[s * IMG_PER_CORE:(s + 1) * IMG_PER_CORE]  # [128, 256, 2, 128]
        arr = shard.reshape(IMG_PER_CORE, NCH, HC, 2, W // 2).transpose(1, 0, 2, 3, 4)
        in_maps.append({"xin": np.ascontiguousarray(arr)})

    res = run_bass_kernel_spmd(nc, in_maps, core_ids=list(range(N_CORES)))

    parts = []
    for s in range(N_CORES):
        o = res.results[s]["out"]  # [NCH, img, 4, 8, 128]
        o = o.transpose(1, 2, 0, 3, 4)  # [img, 4, NCH, 8, 128]
        parts.append(o.reshape(IMG_PER_CORE, 4, H // 2, W // 2))
    full = np.concatenate(parts, axis=0).reshape(B, C, 4, H // 2, W // 2)
    LL = np.ascontiguousarray(full[:, :, 0])
    LH = np.ascontiguousarray(full[:, :, 1])
    HL = np.ascontiguousarray(full[:, :, 2])
    HH = np.ascontiguousarray(full[:, :, 3])
    return (LL, LH, HL, HH)
